# revision 30
# baseline (speedup 1.0000x reference)
"""MoE (brute-force reference) kernel for 8 TRN2 NeuronCores.

Strategy: expert-parallel. Host routes token-slots by gate_idx to their
expert, pads each expert's slot list to capacity C, and transposes so the
device sees xt[e] = X_e.T in a partition-major layout. Each core owns 2
experts and computes
  hT[m] = gelu(sum_k w1T[k,m].T @ xT[k] + b1)   then
  yT[m] = sum_k w2T[k,m].T @ hT[k]
All matmul operands are fp16 (same PE rate as bf16, ~8x the accuracy);
accumulation is fp32 in PSUM. b1 is applied on-device (bias fused into
the gelu activation); b2 and the gate_score combine happen on host in
exact fp32.

Perf notes:
- All tensors are host pre-swizzled into the exact SBUF tile layout
  ([128 partitions, free]) so every DMA is a straight contiguous copy
  with 2-16KB per-partition rows (~350 GB/s busy-rate vs ~250 for the
  strided-slab layout).
- All DMAs ride the two HWDGE rings (no SWDGE/gpsimd: slow descriptor
  generation). The sync ring carries the weight stream in strict
  consumption-deadline order; the scalar ring carries only xt half 0
  early (concurrent with slab 0) plus the y outputs. Extra early DMAs
  on the second ring stall the weight chain via the 8 shared HWDGE
  semaphore lanes (a reused lane serializes its issue on the prior
  DMA's completion), so expert slot 0 uses fine-grained chunks for
  just-in-time arrival and slot 1 (ample lead time) uses coarse chunks
  to minimize lane reuse.
- GEMM2 runs phase A k-outer over k0..7 (earliest-arriving w2 chunks),
  then phase B per-m k-inner over k8..15 so each output completes in
  turn and its eviction + y DMA stream during the remaining matmuls;
  the final output is split across both rings to shorten the tail.
- A ~3.5us dummy-matmul warm-up (vector-engine memset, so it is not
  blocked behind the scalar engine's ACT_TABLE_LOAD) flips the HAM
  clock gate to 8/8 (2.4 GHz) while the first DMAs stream in, so the
  real matmul stream starts at full clock.
"""

import numpy as np

import concourse.bacc as bacc
import concourse.mybir as mybir
from concourse import tile
from concourse.bass_utils import run_bass_kernel_spmd

E, D, H, TOPK, T = 16, 1024, 2048, 2, 2048
NCORES = 8
EPC = E // NCORES  # experts per core
C = 266            # per-expert token capacity after top-k dedup
KD, KH, MD = D // 128, H // 128, D // 128  # 8, 16, 8
HH = H // 2        # GEMM1 column half (m-tiles 0..7 / 8..15)

_F16 = np.float16
_CACHE: dict = {}
_LAST_IN_MAPS = None  # stashed by kernel() for external re-profiling


def _build(reps: int = 1):
    dt = mybir.dt.float16
    f32 = mybir.dt.float32
    nc = bacc.Bacc("TRN2", target_bir_lowering=False, debug=False,
                   num_devices=NCORES)
    # All inputs are pre-swizzled on host to [*, 128, free] so each DMA is
    # a contiguous per-partition copy.
    xt = nc.dram_tensor("xt", [EPC, 128, KD * C], dt, kind="ExternalInput")
    w1 = nc.dram_tensor("w1", [EPC, 128, D * H // 128], dt,
                        kind="ExternalInput")  # 8 slabs(1024) + 2 chunks(4096)
    w2 = nc.dram_tensor("w2", [EPC, 128, H * D // 128], dt,
                        kind="ExternalInput")  # 2 chunks(8192), k-major
    b1 = nc.dram_tensor("b1", [EPC, 128, KH], f32, kind="ExternalInput")
    yt = nc.dram_tensor("yt", [EPC, 128, MD * C], dt, kind="ExternalOutput")

    gelu = mybir.ActivationFunctionType.Gelu_apprx_tanh
    MGRP = 8  # m-tiles per psum group (k-inner within a group)
    WARM = 32  # dummy 128-col matmuls: >=3.4us of sustained PE busy, which
               # flips the HAM clock gate to 8/8 during the warmup itself;
               # the short bridge gap until the first slab lands is safe
               # (re-throttle needs >=3.4us of idle)

    with tile.TileContext(nc) as tc:
        with (
            tc.tile_pool(name="xtp", bufs=2) as xtp,
            tc.tile_pool(name="w1p", bufs=1) as w1p,
            tc.tile_pool(name="w2p", bufs=1) as w2p,
            tc.tile_pool(name="htp", bufs=2) as htp,
            tc.tile_pool(name="yp", bufs=16) as yp,
            tc.tile_pool(name="bp", bufs=2) as bp,
            tc.tile_pool(name="ps", bufs=1, space="PSUM") as psp,
        ):
            # PE warm-up while the first DMAs stream in. memset on the
            # vector engine: the scalar engine is blocked ~1.3us by its
            # ACT_TABLE_LOAD right after the preamble.
            zt = bp.tile([128, 128], dt, name="warmz", tag="warmz")
            nc.vector.memset(zt[:], 0.0)
            psw = psp.tile([128, 128], f32, name="psw", tag="ps7")
            for _ in range(WARM):
                nc.tensor.matmul(psw[:], zt[:], zt[:], start=True, stop=True)

            for r in range(reps):
                for e in range(EPC):
                    u = f"{r}_{e}"

                    # -- sync-ring DMA stream (issue order == service order),
                    # deadline-ordered 1MB chunks so arrival tracks the PE's
                    # just-in-time consumption.
                    hk = KD // 2
                    xth = [xtp.tile([128, hk * C], dt, name=f"xt{u}_{i}",
                                    tag=f"xt{i}") for i in range(2)]

                    # Chunk plans (k-tiles per DMA). Expert slot 0 is
                    # fine-grained so arrival tracks the PE's just-in-time
                    # consumption from a cold start; slot 1 streams with
                    # ample lead, so coarse chunks cut the DMA count and
                    # with it the HWDGE sem-lane reuse stalls.
                    A_PLAN = [1] * KD if e == 0 else [2, 2, 2, 2]
                    B_PLAN = [2, 2, 2, 2] if e == 0 else [4, 4]
                    W2_PLAN = [4, 4, 4, 4] if e == 0 else [8, 8]

                    def chunks(pool, pfx, plan, unit):
                        out, k0 = [], 0
                        for ci, nk in enumerate(plan):
                            tl = pool.tile([128, nk * unit], dt,
                                           name=f"{pfx}{u}_{ci}",
                                           tag=f"{pfx}{e}_{ci}")
                            out.append((tl, k0, nk))
                            k0 += nk
                        return out

                    def cview(chs, k, unit, m):
                        for tl, k0, nk in chs:
                            if k0 <= k < k0 + nk:
                                off = (k - k0) * unit + m * 128
                                return tl[:, off:off + 128]

                    w1a = chunks(w1p, "w1a", A_PLAN, HH)
                    w1b = chunks(w1p, "w1b", B_PLAN, HH)
                    w2c = chunks(w2p, "w2c", W2_PLAN, D)

                    def sdma(tl, k0, nk, dram, base, unit):
                        nc.sync.dma_start(
                            out=tl[:],
                            in_=dram.ap()[e][:, base + k0 * unit:
                                             base + (k0 + nk) * unit])

                    # xt half0 is the ONLY early scalar-ring DMA: it
                    # transfers concurrently with slab 0 without starving
                    # the sync slab chain of the 8 shared HWDGE sem lanes
                    # (each extra early DMA takes a lane; a reused lane
                    # serializes its issue on the prior DMA's completion).
                    # Everything else rides the sync ring in strict
                    # consumption-deadline order.
                    nc.scalar.dma_start(out=xth[0][:],
                                        in_=xt.ap()[e][:, :hk * C])
                    b1s = bp.tile([128, KH], f32, name=f"b1s{u}", tag="b1s")
                    na = len(A_PLAN)
                    for tl, k0, nk in w1a[:na // 2]:
                        sdma(tl, k0, nk, w1, 0, HH)
                    nc.sync.dma_start(out=xth[1][:],
                                      in_=xt.ap()[e][:, hk * C:])
                    for tl, k0, nk in w1a[na // 2:]:
                        sdma(tl, k0, nk, w1, 0, HH)
                    nc.sync.dma_start(out=b1s[:], in_=b1.ap()[e])
                    for tl, k0, nk in w1b:
                        sdma(tl, k0, nk, w1, KD * HH, HH)
                    for tl, k0, nk in w2c:
                        sdma(tl, k0, nk, w2, 0, D)

                    def xtv(k):
                        return xth[k // hk][:, (k % hk) * C:(k % hk + 1) * C]

                    # GEMM1: hT[m] = gelu(sum_k w1[k][:,m].T @ xt[k] + b1)
                    # group 0 (m 0..7) streams per-slab; group 1 uses w1b.
                    hts = [htp.tile([128, C], dt, name=f"ht{u}_{m}",
                                    tag=f"ht{m}") for m in range(KH)]
                    for g in range(0, KH, MGRP):
                        pss = [psp.tile([128, C], f32, name=f"ps1_{u}_{m}",
                                        tag=f"ps{m - g}")
                               for m in range(g, g + MGRP)]
                        for k in range(KD):
                            chs = w1a if g == 0 else w1b
                            for i in range(MGRP):
                                nc.tensor.matmul(
                                    pss[i][:],
                                    cview(chs, k, HH, i),
                                    xtv(k),
                                    start=(k == 0), stop=(k == KD - 1))
                        for i, m in enumerate(range(g, g + MGRP)):
                            nc.scalar.activation(
                                hts[m][:], pss[i][:], gelu,
                                bias=b1s[:, m:m + 1])

                    # GEMM2: yT[m] = sum_k w2[k][:,m].T @ hts[k]
                    # Phase A: k-outer over k 0..7 (earliest-arriving w2
                    # chunks). Phase B: per-m k-inner over k 8..15 so each
                    # m completes in turn and its eviction + y DMA stream
                    # during the remaining matmuls instead of bunching at
                    # the end of the kernel.
                    ps2 = [psp.tile([128, C], f32, name=f"ps2_{u}_{m}",
                                    tag=f"ps{m}") for m in range(MD)]

                    def w2v(k, m):
                        return cview(w2c, k, D, m)

                    for k in range(KH // 2):
                        for m in range(MD):
                            nc.tensor.matmul(ps2[m][:], w2v(k, m), hts[k][:],
                                             start=(k == 0), stop=False)
                    last = (r == reps - 1 and e == EPC - 1)
                    for m in range(MD):
                        for k in range(KH // 2, KH):
                            nc.tensor.matmul(ps2[m][:], w2v(k, m), hts[k][:],
                                             start=False, stop=(k == KH - 1))
                        yo = yp.tile([128, C], dt, name=f"y{u}_{m}", tag="y")
                        if last and m == MD - 1:
                            # Final output: split eviction + DMA across two
                            # engines/rings to shorten the serial tail.
                            CH = C // 2
                            nc.vector.tensor_copy(out=yo[:, :CH],
                                                  in_=ps2[m][:, :CH])
                            nc.scalar.activation(
                                yo[:, CH:], ps2[m][:, CH:],
                                mybir.ActivationFunctionType.Copy)
                            nc.sync.dma_start(
                                out=yt.ap()[e][:, m * C:m * C + CH],
                                in_=yo[:, :CH])
                            nc.scalar.dma_start(
                                out=yt.ap()[e][:, m * C + CH:(m + 1) * C],
                                in_=yo[:, CH:])
                        else:
                            nc.vector.tensor_copy(out=yo[:], in_=ps2[m][:])
                            y_eng = nc.sync if (last and m % 2 == 1) \
                                else nc.scalar
                            y_eng.dma_start(
                                out=yt.ap()[e][:, m * C:(m + 1) * C],
                                in_=yo[:])
    nc.compile()
    return nc


def _get_nc(reps: int = 1):
    if reps not in _CACHE:
        _CACHE[reps] = _build(reps)
    return _CACHE[reps]


def _route(gate_idx, gate_score):
    """Dedup routing: tokens whose two top-k picks are the same expert are
    sent once with summed score. Returns per-expert (tokens, weights,
    overflow_tokens, overflow_weights)."""
    g = np.asarray(gate_idx).astype(np.int64)
    sc = np.asarray(gate_score, dtype=np.float32)
    out = []
    for e in range(E):
        m0, m1 = g[:, 0] == e, g[:, 1] == e
        toks = np.flatnonzero(m0 | m1)
        wts = (sc[:, 0] * m0 + sc[:, 1] * m1)[toks]
        out.append((toks[:C], wts[:C], toks[C:], wts[C:]))
    return out


def kernel(inp, gate_idx, gate_score, w1, b1, w2, b2):
    inp = np.asarray(inp, dtype=np.float32)
    gate_idx = np.asarray(gate_idx)
    gate_score = np.asarray(gate_score, dtype=np.float32)
    w1 = np.asarray(w1, dtype=np.float32)
    b1 = np.asarray(b1, dtype=np.float32)
    w2 = np.asarray(w2, dtype=np.float32)
    b2 = np.asarray(b2, dtype=np.float32)

    routes = _route(gate_idx, gate_score)

    # Host-side gather + swizzle into the device layouts, fp16.
    # xt: [E, 128, KD*C] with [p, k*C+c] = X_e.T[k*128+p, c]
    xt_all = np.zeros((E, 128, KD, C), dtype=_F16)
    for e in range(E):
        toks = routes[e][0]
        n = len(toks)
        if n:
            xt_all[e, :, :, :n] = (
                inp[toks].T.reshape(KD, 128, n).transpose(1, 0, 2)
                .astype(_F16))
    xt_all = xt_all.reshape(E, 128, KD * C)

    # w1: slabs s=0..7 -> w1T[s*128+p, 0:1024]; then 2 chunks j covering
    # k-tiles 4j..4j+3 of columns 1024:2048 (kk-major within a chunk).
    w1t = np.ascontiguousarray(w1.transpose(0, 2, 1)).astype(_F16)  # [E,D,H]
    a = w1t[:, :, :HH].reshape(E, KD, 128, HH).transpose(0, 2, 1, 3)
    b = (w1t[:, :, HH:].reshape(E, 2, 4, 128, HH)
         .transpose(0, 3, 1, 2, 4))
    w1d = np.concatenate(
        [a.reshape(E, 128, KD * HH), b.reshape(E, 128, KD * HH)], axis=2)
    w1d = np.ascontiguousarray(w1d)

    # w2: 2 chunks j covering k-tiles 8j..8j+7 (kk-major), all D columns.
    w2t = np.ascontiguousarray(w2.transpose(0, 2, 1)).astype(_F16)  # [E,H,D]
    w2d = np.ascontiguousarray(
        w2t.reshape(E, 2, 8, 128, D).transpose(0, 3, 1, 2, 4)
        .reshape(E, 128, KH * D))

    in_maps = []
    for c in range(NCORES):
        sl = slice(EPC * c, EPC * (c + 1))
        in_maps.append({
            "xt": xt_all[sl],
            "w1": w1d[sl],
            "w2": w2d[sl],
            "b1": np.ascontiguousarray(
                b1[sl].reshape(EPC, KH, 128).transpose(0, 2, 1)),
        })

    global _LAST_IN_MAPS
    _LAST_IN_MAPS = in_maps

    nc = _get_nc()
    res = run_bass_kernel_spmd(nc, in_maps, list(range(NCORES)))

    # Host combine: weight each expert's output columns by the (summed)
    # gate score and accumulate per token; add the b2 term (folded out of
    # the device kernel). Tokens are unique within an expert, so the
    # fancy-indexed += is safe.
    out = np.einsum("tk,tkd->td", np.asarray(gate_score, dtype=np.float32),
                    b2[np.asarray(gate_idx).astype(np.int64)])
    out = np.ascontiguousarray(out, dtype=np.float32)
    for e in range(E):
        core, le = divmod(e, EPC)
        toks, wts, otoks, owts = routes[e]
        if len(toks):
            yt = res.results[core]["yt"][le].reshape(128, MD, C)
            y = (yt.transpose(1, 0, 2).reshape(D, C)[:, :len(toks)]
                 .T.astype(np.float32))
            out[toks] += wts[:, None] * y
        if len(otoks):  # exact host fallback for capacity overflow
            hh = inp[otoks] @ w1[e].T + b1[e]
            hh = 0.5 * hh * (1.0 + np.tanh(
                np.sqrt(2.0 / np.pi) * (hh + 0.044715 * hh ** 3)))
            out[otoks] += owts[:, None] * (hh @ w2[e].T)
    return out


# revision 33
# speedup vs baseline: 1.0237x; 1.0237x over previous
"""MoE (brute-force reference) kernel for 8 TRN2 NeuronCores.

Strategy: expert-parallel. Host routes token-slots by gate_idx to their
expert, pads each expert's slot list to capacity C, and transposes so the
device sees xt[e] = X_e.T in a partition-major layout. Each core owns 2
experts and computes
  hT[m] = gelu(sum_k w1T[k,m].T @ xT[k] + b1)   then
  yT[m] = sum_k w2T[k,m].T @ hT[k]
All matmul operands are fp16 (same PE rate as bf16, ~8x the accuracy);
accumulation is fp32 in PSUM. b1 is applied on-device (bias fused into
the gelu activation); b2 and the gate_score combine happen on host in
exact fp32.

Perf notes:
- All tensors are host pre-swizzled into the exact SBUF tile layout
  ([128 partitions, free]) so every DMA is a straight contiguous copy
  with 2-16KB per-partition rows (~350 GB/s busy-rate vs ~250 for the
  strided-slab layout).
- All DMAs ride the two HWDGE rings (no SWDGE/gpsimd: slow descriptor
  generation). The sync ring carries the weight stream in strict
  consumption-deadline order; the scalar ring carries only xt half 0
  early (concurrent with slab 0) plus the y outputs. Extra early DMAs
  on the second ring stall the weight chain via the 8 shared HWDGE
  semaphore lanes (a reused lane serializes its issue on the prior
  DMA's completion), so expert slot 0 uses fine-grained chunks for
  just-in-time arrival and slot 1 (ample lead time) uses coarse chunks
  to minimize lane reuse.
- GEMM2 runs phase A k-outer over k0..7 (earliest-arriving w2 chunks),
  then phase B per-m k-inner over k8..15 so each output completes in
  turn and its eviction + y DMA stream during the remaining matmuls.
  The very last output accumulates per column half, so half of it
  evicts and DMAs (on the sync ring) while the other half's matmuls
  still run; the remaining half rides the scalar ring. This cuts the
  serial end-of-kernel chain (eviction -> issue -> transfer) to ~2us.
- A ~3.5us dummy-matmul warm-up (vector-engine memset, so it is not
  blocked behind the scalar engine's ACT_TABLE_LOAD) flips the HAM
  clock gate to 8/8 (2.4 GHz) while the first DMAs stream in, so the
  real matmul stream starts at full clock.
"""

import numpy as np

import concourse.bacc as bacc
import concourse.mybir as mybir
from concourse import tile
from concourse.bass_utils import run_bass_kernel_spmd

E, D, H, TOPK, T = 16, 1024, 2048, 2, 2048
NCORES = 8
EPC = E // NCORES  # experts per core
C = 266            # per-expert token capacity after top-k dedup
KD, KH, MD = D // 128, H // 128, D // 128  # 8, 16, 8
HH = H // 2        # GEMM1 column half (m-tiles 0..7 / 8..15)

_F16 = np.float16
_CACHE: dict = {}
_LAST_IN_MAPS = None  # stashed by kernel() for external re-profiling


def _build(reps: int = 1):
    dt = mybir.dt.float16
    f32 = mybir.dt.float32
    nc = bacc.Bacc("TRN2", target_bir_lowering=False, debug=False,
                   num_devices=NCORES)
    # All inputs are pre-swizzled on host to [*, 128, free] so each DMA is
    # a contiguous per-partition copy.
    xt = nc.dram_tensor("xt", [EPC, 128, KD * C], dt, kind="ExternalInput")
    w1 = nc.dram_tensor("w1", [EPC, 128, D * H // 128], dt,
                        kind="ExternalInput")  # 8 slabs(1024) + 2 chunks(4096)
    w2 = nc.dram_tensor("w2", [EPC, 128, H * D // 128], dt,
                        kind="ExternalInput")  # 2 chunks(8192), k-major
    b1 = nc.dram_tensor("b1", [EPC, 128, KH], f32, kind="ExternalInput")
    yt = nc.dram_tensor("yt", [EPC, 128, MD * C], dt, kind="ExternalOutput")

    gelu = mybir.ActivationFunctionType.Gelu_apprx_tanh
    MGRP = 8  # m-tiles per psum group (k-inner within a group)
    WARM = 32  # dummy 128-col matmuls: >=3.4us of sustained PE busy, which
               # flips the HAM clock gate to 8/8 during the warmup itself;
               # the short bridge gap until the first slab lands is safe
               # (re-throttle needs >=3.4us of idle)

    with tile.TileContext(nc) as tc:
        with (
            tc.tile_pool(name="xtp", bufs=2) as xtp,
            tc.tile_pool(name="w1p", bufs=1) as w1p,
            tc.tile_pool(name="w2p", bufs=1) as w2p,
            tc.tile_pool(name="htp", bufs=2) as htp,
            tc.tile_pool(name="yp", bufs=16) as yp,
            tc.tile_pool(name="bp", bufs=2) as bp,
            tc.tile_pool(name="ps", bufs=1, space="PSUM") as psp,
        ):
            # PE warm-up while the first DMAs stream in. memset on the
            # vector engine: the scalar engine is blocked ~1.3us by its
            # ACT_TABLE_LOAD right after the preamble.
            zt = bp.tile([128, 128], dt, name="warmz", tag="warmz")
            nc.vector.memset(zt[:], 0.0)
            psw = psp.tile([128, 128], f32, name="psw", tag="ps7")
            for _ in range(WARM):
                nc.tensor.matmul(psw[:], zt[:], zt[:], start=True, stop=True)

            for r in range(reps):
                for e in range(EPC):
                    u = f"{r}_{e}"

                    # -- sync-ring DMA stream (issue order == service order),
                    # deadline-ordered 1MB chunks so arrival tracks the PE's
                    # just-in-time consumption.
                    hk = KD // 2
                    xth = [xtp.tile([128, hk * C], dt, name=f"xt{u}_{i}",
                                    tag=f"xt{i}") for i in range(2)]

                    # Chunk plans (k-tiles per DMA). Expert slot 0 is
                    # fine-grained so arrival tracks the PE's just-in-time
                    # consumption from a cold start; slot 1 streams with
                    # ample lead, so coarse chunks cut the DMA count and
                    # with it the HWDGE sem-lane reuse stalls.
                    # Slot 0's late chunks coarsen too ([2,2,4]/[4,4,8]):
                    # their deadlines have slack, and two fewer DMAs free
                    # sem lanes earlier for slot 1's stream.
                    A_PLAN = [1] * KD if e == 0 else [2, 2, 2, 2]
                    B_PLAN = [2, 2, 4] if e == 0 else [4, 4]
                    W2_PLAN = [4, 4, 8] if e == 0 else [8, 8]

                    def chunks(pool, pfx, plan, unit):
                        out, k0 = [], 0
                        for ci, nk in enumerate(plan):
                            tl = pool.tile([128, nk * unit], dt,
                                           name=f"{pfx}{u}_{ci}",
                                           tag=f"{pfx}{e}_{ci}")
                            out.append((tl, k0, nk))
                            k0 += nk
                        return out

                    def cview(chs, k, unit, m):
                        for tl, k0, nk in chs:
                            if k0 <= k < k0 + nk:
                                off = (k - k0) * unit + m * 128
                                return tl[:, off:off + 128]

                    w1a = chunks(w1p, "w1a", A_PLAN, HH)
                    w1b = chunks(w1p, "w1b", B_PLAN, HH)
                    w2c = chunks(w2p, "w2c", W2_PLAN, D)

                    def sdma(tl, k0, nk, dram, base, unit):
                        nc.sync.dma_start(
                            out=tl[:],
                            in_=dram.ap()[e][:, base + k0 * unit:
                                             base + (k0 + nk) * unit])

                    # xt half0 is the ONLY early scalar-ring DMA: it
                    # transfers concurrently with slab 0 without starving
                    # the sync slab chain of the 8 shared HWDGE sem lanes
                    # (each extra early DMA takes a lane; a reused lane
                    # serializes its issue on the prior DMA's completion).
                    # Everything else rides the sync ring in strict
                    # consumption-deadline order.
                    nc.scalar.dma_start(out=xth[0][:],
                                        in_=xt.ap()[e][:, :hk * C])
                    b1s = bp.tile([128, KH], f32, name=f"b1s{u}", tag="b1s")
                    na = len(A_PLAN)
                    for tl, k0, nk in w1a[:na // 2]:
                        sdma(tl, k0, nk, w1, 0, HH)
                    nc.sync.dma_start(out=xth[1][:],
                                      in_=xt.ap()[e][:, hk * C:])
                    for tl, k0, nk in w1a[na // 2:]:
                        sdma(tl, k0, nk, w1, 0, HH)
                    nc.sync.dma_start(out=b1s[:], in_=b1.ap()[e])
                    for tl, k0, nk in w1b:
                        sdma(tl, k0, nk, w1, KD * HH, HH)
                    for tl, k0, nk in w2c:
                        sdma(tl, k0, nk, w2, 0, D)

                    def xtv(k):
                        return xth[k // hk][:, (k % hk) * C:(k % hk + 1) * C]

                    # GEMM1: hT[m] = gelu(sum_k w1[k][:,m].T @ xt[k] + b1)
                    # group 0 (m 0..7) streams per-slab; group 1 uses w1b.
                    hts = [htp.tile([128, C], dt, name=f"ht{u}_{m}",
                                    tag=f"ht{m}") for m in range(KH)]
                    for g in range(0, KH, MGRP):
                        pss = [psp.tile([128, C], f32, name=f"ps1_{u}_{m}",
                                        tag=f"ps{m - g}")
                               for m in range(g, g + MGRP)]
                        for k in range(KD):
                            chs = w1a if g == 0 else w1b
                            for i in range(MGRP):
                                nc.tensor.matmul(
                                    pss[i][:],
                                    cview(chs, k, HH, i),
                                    xtv(k),
                                    start=(k == 0), stop=(k == KD - 1))
                        for i, m in enumerate(range(g, g + MGRP)):
                            nc.scalar.activation(
                                hts[m][:], pss[i][:], gelu,
                                bias=b1s[:, m:m + 1])

                    # GEMM2: yT[m] = sum_k w2[k][:,m].T @ hts[k]
                    # Phase A: k-outer over k 0..7 (earliest-arriving w2
                    # chunks). Phase B: per-m k-inner over k 8..15 so each
                    # m completes in turn and its eviction + y DMA stream
                    # during the remaining matmuls instead of bunching at
                    # the end of the kernel.
                    ps2 = [psp.tile([128, C], f32, name=f"ps2_{u}_{m}",
                                    tag=f"ps{m}") for m in range(MD)]

                    def w2v(k, m):
                        return cview(w2c, k, D, m)

                    for k in range(KH // 2):
                        for m in range(MD):
                            nc.tensor.matmul(ps2[m][:], w2v(k, m), hts[k][:],
                                             start=(k == 0), stop=False)
                    last = (r == reps - 1 and e == EPC - 1)
                    for m in range(MD):
                        yo = yp.tile([128, C], dt, name=f"y{u}_{m}", tag="y")
                        if last and m == MD - 1:
                            # Final output: accumulate k8..15 per column
                            # half so the first half's eviction + DMA run
                            # while the second half's matmuls are still on
                            # the PE, then split the remaining tail across
                            # both engines/rings.
                            CH = C // 2
                            for k in range(KH // 2, KH):
                                nc.tensor.matmul(
                                    ps2[m][:, :CH], w2v(k, m),
                                    hts[k][:, :CH],
                                    start=False, stop=(k == KH - 1))
                            nc.vector.tensor_copy(out=yo[:, :CH],
                                                  in_=ps2[m][:, :CH])
                            nc.sync.dma_start(
                                out=yt.ap()[e][:, m * C:m * C + CH],
                                in_=yo[:, :CH])
                            for k in range(KH // 2, KH):
                                nc.tensor.matmul(
                                    ps2[m][:, CH:], w2v(k, m),
                                    hts[k][:, CH:],
                                    start=False, stop=(k == KH - 1))
                            nc.scalar.activation(
                                yo[:, CH:], ps2[m][:, CH:],
                                mybir.ActivationFunctionType.Copy)
                            nc.scalar.dma_start(
                                out=yt.ap()[e][:, m * C + CH:(m + 1) * C],
                                in_=yo[:, CH:])
                        else:
                            for k in range(KH // 2, KH):
                                nc.tensor.matmul(
                                    ps2[m][:], w2v(k, m), hts[k][:],
                                    start=False, stop=(k == KH - 1))
                            nc.vector.tensor_copy(out=yo[:], in_=ps2[m][:])
                            y_eng = nc.sync if (last and m % 2 == 1) \
                                else nc.scalar
                            y_eng.dma_start(
                                out=yt.ap()[e][:, m * C:(m + 1) * C],
                                in_=yo[:])
    nc.compile()
    return nc


def _get_nc(reps: int = 1):
    if reps not in _CACHE:
        _CACHE[reps] = _build(reps)
    return _CACHE[reps]


def _route(gate_idx, gate_score):
    """Dedup routing: tokens whose two top-k picks are the same expert are
    sent once with summed score. Returns per-expert (tokens, weights,
    overflow_tokens, overflow_weights)."""
    g = np.asarray(gate_idx).astype(np.int64)
    sc = np.asarray(gate_score, dtype=np.float32)
    out = []
    for e in range(E):
        m0, m1 = g[:, 0] == e, g[:, 1] == e
        toks = np.flatnonzero(m0 | m1)
        wts = (sc[:, 0] * m0 + sc[:, 1] * m1)[toks]
        out.append((toks[:C], wts[:C], toks[C:], wts[C:]))
    return out


def kernel(inp, gate_idx, gate_score, w1, b1, w2, b2):
    inp = np.asarray(inp, dtype=np.float32)
    gate_idx = np.asarray(gate_idx)
    gate_score = np.asarray(gate_score, dtype=np.float32)
    w1 = np.asarray(w1, dtype=np.float32)
    b1 = np.asarray(b1, dtype=np.float32)
    w2 = np.asarray(w2, dtype=np.float32)
    b2 = np.asarray(b2, dtype=np.float32)

    routes = _route(gate_idx, gate_score)

    # Host-side gather + swizzle into the device layouts, fp16.
    # xt: [E, 128, KD*C] with [p, k*C+c] = X_e.T[k*128+p, c]
    xt_all = np.zeros((E, 128, KD, C), dtype=_F16)
    for e in range(E):
        toks = routes[e][0]
        n = len(toks)
        if n:
            xt_all[e, :, :, :n] = (
                inp[toks].T.reshape(KD, 128, n).transpose(1, 0, 2)
                .astype(_F16))
    xt_all = xt_all.reshape(E, 128, KD * C)

    # w1: slabs s=0..7 -> w1T[s*128+p, 0:1024]; then 2 chunks j covering
    # k-tiles 4j..4j+3 of columns 1024:2048 (kk-major within a chunk).
    w1t = np.ascontiguousarray(w1.transpose(0, 2, 1)).astype(_F16)  # [E,D,H]
    a = w1t[:, :, :HH].reshape(E, KD, 128, HH).transpose(0, 2, 1, 3)
    b = (w1t[:, :, HH:].reshape(E, 2, 4, 128, HH)
         .transpose(0, 3, 1, 2, 4))
    w1d = np.concatenate(
        [a.reshape(E, 128, KD * HH), b.reshape(E, 128, KD * HH)], axis=2)
    w1d = np.ascontiguousarray(w1d)

    # w2: 2 chunks j covering k-tiles 8j..8j+7 (kk-major), all D columns.
    w2t = np.ascontiguousarray(w2.transpose(0, 2, 1)).astype(_F16)  # [E,H,D]
    w2d = np.ascontiguousarray(
        w2t.reshape(E, 2, 8, 128, D).transpose(0, 3, 1, 2, 4)
        .reshape(E, 128, KH * D))

    in_maps = []
    for c in range(NCORES):
        sl = slice(EPC * c, EPC * (c + 1))
        in_maps.append({
            "xt": xt_all[sl],
            "w1": w1d[sl],
            "w2": w2d[sl],
            "b1": np.ascontiguousarray(
                b1[sl].reshape(EPC, KH, 128).transpose(0, 2, 1)),
        })

    global _LAST_IN_MAPS
    _LAST_IN_MAPS = in_maps

    nc = _get_nc()
    res = run_bass_kernel_spmd(nc, in_maps, list(range(NCORES)))

    # Host combine: weight each expert's output columns by the (summed)
    # gate score and accumulate per token; add the b2 term (folded out of
    # the device kernel). Tokens are unique within an expert, so the
    # fancy-indexed += is safe.
    out = np.einsum("tk,tkd->td", np.asarray(gate_score, dtype=np.float32),
                    b2[np.asarray(gate_idx).astype(np.int64)])
    out = np.ascontiguousarray(out, dtype=np.float32)
    for e in range(E):
        core, le = divmod(e, EPC)
        toks, wts, otoks, owts = routes[e]
        if len(toks):
            yt = res.results[core]["yt"][le].reshape(128, MD, C)
            y = (yt.transpose(1, 0, 2).reshape(D, C)[:, :len(toks)]
                 .T.astype(np.float32))
            out[toks] += wts[:, None] * y
        if len(otoks):  # exact host fallback for capacity overflow
            hh = inp[otoks] @ w1[e].T + b1[e]
            hh = 0.5 * hh * (1.0 + np.tanh(
                np.sqrt(2.0 / np.pi) * (hh + 0.044715 * hh ** 3)))
            out[otoks] += owts[:, None] * (hh @ w2[e].T)
    return out


# revision 34
# speedup vs baseline: 1.0317x; 1.0078x over previous
"""MoE (brute-force reference) kernel for 8 TRN2 NeuronCores.

Strategy: expert-parallel. Host routes token-slots by gate_idx to their
expert, pads each expert's slot list to capacity C, and transposes so the
device sees xt[e] = X_e.T in a partition-major layout. Each core owns 2
experts and computes
  hT[m] = gelu(sum_k w1T[k,m].T @ xT[k] + b1)   then
  yT[m] = sum_k w2T[k,m].T @ hT[k]
All matmul operands are fp16 (same PE rate as bf16, ~8x the accuracy);
accumulation is fp32 in PSUM. b1 is applied on-device (bias fused into
the gelu activation); b2 and the gate_score combine happen on host in
exact fp32.

Perf notes:
- All tensors are host pre-swizzled into the exact SBUF tile layout
  ([128 partitions, free]) so every DMA is a straight contiguous copy
  with 2-16KB per-partition rows (~350 GB/s busy-rate vs ~250 for the
  strided-slab layout).
- All DMAs ride the two HWDGE rings (no SWDGE/gpsimd: slow descriptor
  generation). The sync ring carries the weight stream in strict
  consumption-deadline order; the scalar ring carries only xt half 0
  early (concurrent with slab 0) plus the y outputs. Extra early DMAs
  on the second ring stall the weight chain via the 8 shared HWDGE
  semaphore lanes (a reused lane serializes its issue on the prior
  DMA's completion), so expert slot 0 uses fine-grained chunks for
  just-in-time arrival and slot 1 (ample lead time) uses coarse chunks
  to minimize lane reuse.
- GEMM2 runs phase A k-outer over k0..7 (earliest-arriving w2 chunks),
  then phase B per-m k-inner over k8..15 so each output completes in
  turn and its eviction + y DMA stream during the remaining matmuls.
  The very last output accumulates per column half, so half of it
  evicts and DMAs (on the sync ring) while the other half's matmuls
  still run; the remaining half rides the scalar ring. This cuts the
  serial end-of-kernel chain (eviction -> issue -> transfer) to ~2us.
- A ~3.5us dummy-matmul warm-up (vector-engine memset, so it is not
  blocked behind the scalar engine's ACT_TABLE_LOAD) flips the HAM
  clock gate to 8/8 (2.4 GHz) while the first DMAs stream in, so the
  real matmul stream starts at full clock.
"""

import numpy as np

import concourse.bacc as bacc
import concourse.mybir as mybir
from concourse import tile
from concourse.bass_utils import run_bass_kernel_spmd

E, D, H, TOPK, T = 16, 1024, 2048, 2, 2048
NCORES = 8
EPC = E // NCORES  # experts per core
C = 266            # per-expert token capacity after top-k dedup
KD, KH, MD = D // 128, H // 128, D // 128  # 8, 16, 8
HH = H // 2        # GEMM1 column half (m-tiles 0..7 / 8..15)

_F16 = np.float16
_CACHE: dict = {}
_LAST_IN_MAPS = None  # stashed by kernel() for external re-profiling


def _build(reps: int = 1):
    dt = mybir.dt.float16
    f32 = mybir.dt.float32
    nc = bacc.Bacc("TRN2", target_bir_lowering=False, debug=False,
                   num_devices=NCORES)
    # All inputs are pre-swizzled on host to [*, 128, free] so each DMA is
    # a contiguous per-partition copy.
    xt = nc.dram_tensor("xt", [EPC, 128, KD * C], dt, kind="ExternalInput")
    w1 = nc.dram_tensor("w1", [EPC, 128, D * H // 128], dt,
                        kind="ExternalInput")  # 8 slabs(1024) + 2 chunks(4096)
    w2 = nc.dram_tensor("w2", [EPC, 128, H * D // 128], dt,
                        kind="ExternalInput")  # 2 chunks(8192), k-major
    b1 = nc.dram_tensor("b1", [EPC, 128, KH], f32, kind="ExternalInput")
    yt = nc.dram_tensor("yt", [EPC, 128, MD * C], dt, kind="ExternalOutput")

    gelu = mybir.ActivationFunctionType.Gelu_apprx_tanh
    MGRP = 8  # m-tiles per psum group (k-inner within a group)
    WARM = 32  # dummy 128-col matmuls: >=3.4us of sustained PE busy, which
               # flips the HAM clock gate to 8/8 during the warmup itself;
               # the short bridge gap until the first slab lands is safe
               # (re-throttle needs >=3.4us of idle)

    with tile.TileContext(nc) as tc:
        with (
            # bufs=1: expert 1's xt DMA then waits for expert 0's last
            # GEMM1 read of the shared tile tag (~25us) instead of being
            # hoisted to t=7.5us by the scheduler, where its transfer
            # steals HBM bandwidth from the critical slab0+xt0 window.
            tc.tile_pool(name="xtp", bufs=1) as xtp,
            tc.tile_pool(name="w1p", bufs=1) as w1p,
            tc.tile_pool(name="w2p", bufs=1) as w2p,
            tc.tile_pool(name="htp", bufs=2) as htp,
            tc.tile_pool(name="yp", bufs=16) as yp,
            tc.tile_pool(name="bp", bufs=2) as bp,
            tc.tile_pool(name="ps", bufs=1, space="PSUM") as psp,
        ):
            # PE warm-up while the first DMAs stream in. memset on the
            # vector engine: the scalar engine is blocked ~1.3us by its
            # ACT_TABLE_LOAD right after the preamble.
            zt = bp.tile([128, 128], dt, name="warmz", tag="warmz")
            nc.vector.memset(zt[:], 0.0)
            psw = psp.tile([128, 128], f32, name="psw", tag="ps7")
            for _ in range(WARM):
                nc.tensor.matmul(psw[:], zt[:], zt[:], start=True, stop=True)

            for r in range(reps):
                for e in range(EPC):
                    u = f"{r}_{e}"

                    # -- sync-ring DMA stream (issue order == service order),
                    # deadline-ordered 1MB chunks so arrival tracks the PE's
                    # just-in-time consumption.
                    hk = KD // 2
                    xth = [xtp.tile([128, hk * C], dt, name=f"xt{u}_{i}",
                                    tag=f"xt{i}") for i in range(2)]

                    # Chunk plans (k-tiles per DMA). Expert slot 0 is
                    # fine-grained so arrival tracks the PE's just-in-time
                    # consumption from a cold start; slot 1 streams with
                    # ample lead, so coarse chunks cut the DMA count and
                    # with it the HWDGE sem-lane reuse stalls.
                    # Slot 0's late chunks coarsen too ([2,2,4]/[4,4,8]):
                    # their deadlines have slack, and two fewer DMAs free
                    # sem lanes earlier for slot 1's stream.
                    A_PLAN = [1] * KD if e == 0 else [2, 2, 2, 2]
                    B_PLAN = [2, 2, 4] if e == 0 else [4, 4]
                    W2_PLAN = [4, 4, 8] if e == 0 else [8, 8]

                    def chunks(pool, pfx, plan, unit):
                        out, k0 = [], 0
                        for ci, nk in enumerate(plan):
                            tl = pool.tile([128, nk * unit], dt,
                                           name=f"{pfx}{u}_{ci}",
                                           tag=f"{pfx}{e}_{ci}")
                            out.append((tl, k0, nk))
                            k0 += nk
                        return out

                    def cview(chs, k, unit, m):
                        for tl, k0, nk in chs:
                            if k0 <= k < k0 + nk:
                                off = (k - k0) * unit + m * 128
                                return tl[:, off:off + 128]

                    w1a = chunks(w1p, "w1a", A_PLAN, HH)
                    w1b = chunks(w1p, "w1b", B_PLAN, HH)
                    w2c = chunks(w2p, "w2c", W2_PLAN, D)

                    def sdma(tl, k0, nk, dram, base, unit):
                        nc.sync.dma_start(
                            out=tl[:],
                            in_=dram.ap()[e][:, base + k0 * unit:
                                             base + (k0 + nk) * unit])

                    # xt half0 is the ONLY early scalar-ring DMA: it
                    # transfers concurrently with slab 0 without starving
                    # the sync slab chain of the 8 shared HWDGE sem lanes
                    # (each extra early DMA takes a lane; a reused lane
                    # serializes its issue on the prior DMA's completion).
                    # Everything else rides the sync ring in strict
                    # consumption-deadline order.
                    nc.scalar.dma_start(out=xth[0][:],
                                        in_=xt.ap()[e][:, :hk * C])
                    b1s = bp.tile([128, KH], f32, name=f"b1s{u}", tag="b1s")
                    na = len(A_PLAN)
                    for tl, k0, nk in w1a[:na // 2]:
                        sdma(tl, k0, nk, w1, 0, HH)
                    nc.sync.dma_start(out=xth[1][:],
                                      in_=xt.ap()[e][:, hk * C:])
                    for tl, k0, nk in w1a[na // 2:]:
                        sdma(tl, k0, nk, w1, 0, HH)
                    nc.sync.dma_start(out=b1s[:], in_=b1.ap()[e])
                    for tl, k0, nk in w1b:
                        sdma(tl, k0, nk, w1, KD * HH, HH)
                    for tl, k0, nk in w2c:
                        sdma(tl, k0, nk, w2, 0, D)

                    def xtv(k):
                        return xth[k // hk][:, (k % hk) * C:(k % hk + 1) * C]

                    # GEMM1: hT[m] = gelu(sum_k w1[k][:,m].T @ xt[k] + b1)
                    # group 0 (m 0..7) streams per-slab; group 1 uses w1b.
                    hts = [htp.tile([128, C], dt, name=f"ht{u}_{m}",
                                    tag=f"ht{m}") for m in range(KH)]
                    for g in range(0, KH, MGRP):
                        pss = [psp.tile([128, C], f32, name=f"ps1_{u}_{m}",
                                        tag=f"ps{m - g}")
                               for m in range(g, g + MGRP)]
                        for k in range(KD):
                            chs = w1a if g == 0 else w1b
                            for i in range(MGRP):
                                nc.tensor.matmul(
                                    pss[i][:],
                                    cview(chs, k, HH, i),
                                    xtv(k),
                                    start=(k == 0), stop=(k == KD - 1))
                        for i, m in enumerate(range(g, g + MGRP)):
                            nc.scalar.activation(
                                hts[m][:], pss[i][:], gelu,
                                bias=b1s[:, m:m + 1])

                    # GEMM2: yT[m] = sum_k w2[k][:,m].T @ hts[k]
                    # Phase A: k-outer over k 0..7 (earliest-arriving w2
                    # chunks). Phase B: per-m k-inner over k 8..15 so each
                    # m completes in turn and its eviction + y DMA stream
                    # during the remaining matmuls instead of bunching at
                    # the end of the kernel.
                    ps2 = [psp.tile([128, C], f32, name=f"ps2_{u}_{m}",
                                    tag=f"ps{m}") for m in range(MD)]

                    def w2v(k, m):
                        return cview(w2c, k, D, m)

                    for k in range(KH // 2):
                        for m in range(MD):
                            nc.tensor.matmul(ps2[m][:], w2v(k, m), hts[k][:],
                                             start=(k == 0), stop=False)
                    last = (r == reps - 1 and e == EPC - 1)
                    for m in range(MD):
                        yo = yp.tile([128, C], dt, name=f"y{u}_{m}", tag="y")
                        if last and m == MD - 1:
                            # Final output: accumulate k8..15 per column
                            # half so the first half's eviction + DMA run
                            # while the second half's matmuls are still on
                            # the PE, then split the remaining tail across
                            # both engines/rings.
                            CH = C // 2
                            for k in range(KH // 2, KH):
                                nc.tensor.matmul(
                                    ps2[m][:, :CH], w2v(k, m),
                                    hts[k][:, :CH],
                                    start=False, stop=(k == KH - 1))
                            nc.vector.tensor_copy(out=yo[:, :CH],
                                                  in_=ps2[m][:, :CH])
                            nc.sync.dma_start(
                                out=yt.ap()[e][:, m * C:m * C + CH],
                                in_=yo[:, :CH])
                            for k in range(KH // 2, KH):
                                nc.tensor.matmul(
                                    ps2[m][:, CH:], w2v(k, m),
                                    hts[k][:, CH:],
                                    start=False, stop=(k == KH - 1))
                            nc.scalar.activation(
                                yo[:, CH:], ps2[m][:, CH:],
                                mybir.ActivationFunctionType.Copy)
                            nc.scalar.dma_start(
                                out=yt.ap()[e][:, m * C + CH:(m + 1) * C],
                                in_=yo[:, CH:])
                        else:
                            for k in range(KH // 2, KH):
                                nc.tensor.matmul(
                                    ps2[m][:], w2v(k, m), hts[k][:],
                                    start=False, stop=(k == KH - 1))
                            nc.vector.tensor_copy(out=yo[:], in_=ps2[m][:])
                            y_eng = nc.sync if (last and m % 2 == 1) \
                                else nc.scalar
                            y_eng.dma_start(
                                out=yt.ap()[e][:, m * C:(m + 1) * C],
                                in_=yo[:])
    nc.compile()
    return nc


def _get_nc(reps: int = 1):
    if reps not in _CACHE:
        _CACHE[reps] = _build(reps)
    return _CACHE[reps]


def _route(gate_idx, gate_score):
    """Dedup routing: tokens whose two top-k picks are the same expert are
    sent once with summed score. Returns per-expert (tokens, weights,
    overflow_tokens, overflow_weights)."""
    g = np.asarray(gate_idx).astype(np.int64)
    sc = np.asarray(gate_score, dtype=np.float32)
    out = []
    for e in range(E):
        m0, m1 = g[:, 0] == e, g[:, 1] == e
        toks = np.flatnonzero(m0 | m1)
        wts = (sc[:, 0] * m0 + sc[:, 1] * m1)[toks]
        out.append((toks[:C], wts[:C], toks[C:], wts[C:]))
    return out


def kernel(inp, gate_idx, gate_score, w1, b1, w2, b2):
    inp = np.asarray(inp, dtype=np.float32)
    gate_idx = np.asarray(gate_idx)
    gate_score = np.asarray(gate_score, dtype=np.float32)
    w1 = np.asarray(w1, dtype=np.float32)
    b1 = np.asarray(b1, dtype=np.float32)
    w2 = np.asarray(w2, dtype=np.float32)
    b2 = np.asarray(b2, dtype=np.float32)

    routes = _route(gate_idx, gate_score)

    # Host-side gather + swizzle into the device layouts, fp16.
    # xt: [E, 128, KD*C] with [p, k*C+c] = X_e.T[k*128+p, c]
    xt_all = np.zeros((E, 128, KD, C), dtype=_F16)
    for e in range(E):
        toks = routes[e][0]
        n = len(toks)
        if n:
            xt_all[e, :, :, :n] = (
                inp[toks].T.reshape(KD, 128, n).transpose(1, 0, 2)
                .astype(_F16))
    xt_all = xt_all.reshape(E, 128, KD * C)

    # w1: slabs s=0..7 -> w1T[s*128+p, 0:1024]; then 2 chunks j covering
    # k-tiles 4j..4j+3 of columns 1024:2048 (kk-major within a chunk).
    w1t = np.ascontiguousarray(w1.transpose(0, 2, 1)).astype(_F16)  # [E,D,H]
    a = w1t[:, :, :HH].reshape(E, KD, 128, HH).transpose(0, 2, 1, 3)
    b = (w1t[:, :, HH:].reshape(E, 2, 4, 128, HH)
         .transpose(0, 3, 1, 2, 4))
    w1d = np.concatenate(
        [a.reshape(E, 128, KD * HH), b.reshape(E, 128, KD * HH)], axis=2)
    w1d = np.ascontiguousarray(w1d)

    # w2: 2 chunks j covering k-tiles 8j..8j+7 (kk-major), all D columns.
    w2t = np.ascontiguousarray(w2.transpose(0, 2, 1)).astype(_F16)  # [E,H,D]
    w2d = np.ascontiguousarray(
        w2t.reshape(E, 2, 8, 128, D).transpose(0, 3, 1, 2, 4)
        .reshape(E, 128, KH * D))

    in_maps = []
    for c in range(NCORES):
        sl = slice(EPC * c, EPC * (c + 1))
        in_maps.append({
            "xt": xt_all[sl],
            "w1": w1d[sl],
            "w2": w2d[sl],
            "b1": np.ascontiguousarray(
                b1[sl].reshape(EPC, KH, 128).transpose(0, 2, 1)),
        })

    global _LAST_IN_MAPS
    _LAST_IN_MAPS = in_maps

    nc = _get_nc()
    res = run_bass_kernel_spmd(nc, in_maps, list(range(NCORES)))

    # Host combine: weight each expert's output columns by the (summed)
    # gate score and accumulate per token; add the b2 term (folded out of
    # the device kernel). Tokens are unique within an expert, so the
    # fancy-indexed += is safe.
    out = np.einsum("tk,tkd->td", np.asarray(gate_score, dtype=np.float32),
                    b2[np.asarray(gate_idx).astype(np.int64)])
    out = np.ascontiguousarray(out, dtype=np.float32)
    for e in range(E):
        core, le = divmod(e, EPC)
        toks, wts, otoks, owts = routes[e]
        if len(toks):
            yt = res.results[core]["yt"][le].reshape(128, MD, C)
            y = (yt.transpose(1, 0, 2).reshape(D, C)[:, :len(toks)]
                 .T.astype(np.float32))
            out[toks] += wts[:, None] * y
        if len(otoks):  # exact host fallback for capacity overflow
            hh = inp[otoks] @ w1[e].T + b1[e]
            hh = 0.5 * hh * (1.0 + np.tanh(
                np.sqrt(2.0 / np.pi) * (hh + 0.044715 * hh ** 3)))
            out[otoks] += owts[:, None] * (hh @ w2[e].T)
    return out
